# revision 21
# baseline (speedup 1.0000x reference)
"""MoE FFN (top-2 of 8 experts, SwiGLU) for 8 Trainium2 NeuronCores.

Strategy: expert parallelism. The router (tiny [T,H]@[H,E] matmul + softmax +
top-2) runs on host as part of sharding; tokens are dispatched ("alltoall by
routing decision") to the core owning their expert. Each core runs a dense
SwiGLU FFN over its gathered tokens in bf16 (fp32 PSUM accumulation), in a
feature-on-partition / token-on-free-dim layout so no on-device transposes are
needed and every weight byte is DMA'd exactly once, as a handful of large
contiguous transfers. The host applies the combine weights and scatter-adds
the per-expert outputs into the full output.

Per-core device program (expert e), with nht = H/128 h-tiles, f-chunks of
FCH columns (nft f-tiles each):
  g_T[f, t] = sum_i w1[h_i, f]^T @ x_T[h_i, t]        (PSUM accum over h-tiles)
  u_T[f, t] likewise with w2
  h_T[f, t] = silu(g_T + b1) * (u_T + b2)             (ACT + DVE, -> bf16)
  y_T[h, t] = sum_f w3[f, h]^T @ h_T[f, t] + b3       (PSUM accum per f-chunk,
                                                       accumulated in SBUF f32)
Weights stream through SBUF one f-chunk at a time; tokens/outputs are SBUF-
resident. Every matmul has a 128-row stationary operand in natural layout and
a [128, block] moving operand, so the PE runs back-to-back at stream rate.
"""

import numpy as np
import ml_dtypes

E = 8       # experts == cores
K = 2       # top-k
H = 1024    # hidden
F = 4096    # ffn dim
BLK = 512   # max tokens per block (moving free dim of every matmul)
FCH = 512   # f-chunk size (weight streaming granularity); FCH % 128 == 0

NHT = H // 128    # h-tiles
NFCH = F // FCH   # f-chunks
NFT = FCH // 128  # f-tiles per chunk

_BF16 = ml_dtypes.bfloat16

_kernel_cache: dict[object, object] = {}
_last_in_maps = None


def _blocks_for(max_n: int):
    """Token-block sizes covering max_n tokens: full 512-blocks plus a small
    tail block, so padded capacity hugs the real max expert load."""
    max_n = max(max_n, 16)
    nfull, rem = divmod(max_n, BLK)
    rem = (rem + 7) // 8 * 8  # keep DMA rows 16B-aligned
    sizes = [BLK] * nfull + ([rem] if rem else [])
    blocks = []
    off = 0
    for sz in sizes:
        blocks.append((off, sz))
        off += sz
    return blocks, off


def _build(blocks, use_b2: bool):
    """Build the per-core Bass/Tile program for the given token blocks."""
    import concourse.bass as bass  # noqa: F401
    import concourse.tile as tile
    from concourse import bacc, mybir

    bf16 = mybir.dt.bfloat16
    f32 = mybir.dt.float32
    AF = mybir.ActivationFunctionType

    cap = sum(sz for _, sz in blocks)

    nc = bacc.Bacc("TRN2", target_bir_lowering=False, debug=False, num_devices=E)

    # Host-side layouts are chosen so every DMA is a large 2D/3D transfer with
    # long contiguous rows (see kernel() for the packing).
    xT = nc.declare_dram_parameter("xT", [128, NHT * cap], bf16, isOutput=False)
    w1 = nc.declare_dram_parameter("w1", [NFCH, 128, NFT * H], bf16, isOutput=False)
    w2 = nc.declare_dram_parameter("w2", [NFCH, 128, NFT * H], bf16, isOutput=False)
    w3 = nc.declare_dram_parameter("w3", [NFCH, 128, NFT * H], bf16, isOutput=False)
    b1 = nc.declare_dram_parameter("b1", [128, F // 128], f32, isOutput=False)
    b3 = nc.declare_dram_parameter("b3", [128, NHT], f32, isOutput=False)
    if use_b2:
        b2 = nc.declare_dram_parameter("b2", [128, F // 128], f32, isOutput=False)
    yT = nc.declare_dram_parameter("yT", [128, NHT * cap], f32, isOutput=True)

    with tile.TileContext(nc) as tc:
        with (
            tc.tile_pool(name="xp", bufs=1) as xp,
            tc.tile_pool(name="yp", bufs=1) as yp,
            tc.tile_pool(name="wp", bufs=2) as wp,
            tc.tile_pool(name="hp", bufs=2) as hp,
            tc.tile_pool(name="sp", bufs=3) as sp,
            tc.tile_pool(name="bp", bufs=1) as bp,
            tc.tile_pool(name="pg", bufs=2, space="PSUM") as pg,
            tc.tile_pool(name="pu", bufs=2, space="PSUM") as pu,
            tc.tile_pool(name="py", bufs=2, space="PSUM") as py,
        ):
            # Biases (tiny, resident)
            b1t = bp.tile([128, F // 128], f32, tag="b1", name="b1t")
            nc.gpsimd.dma_start(b1t[:], b1[:])
            b3t = bp.tile([128, NHT], f32, tag="b3", name="b3t")
            nc.gpsimd.dma_start(b3t[:], b3[:])
            if use_b2:
                b2t = bp.tile([128, F // 128], f32, tag="b2", name="b2t")
                nc.gpsimd.dma_start(b2t[:], b2[:])

            # Tokens (resident, bf16): one [128, NHT*cap] tile in BLOCK-major
            # column order — token block at offset `off` occupies columns
            # [NHT*off, NHT*(off+sz)), h-tile i contiguous inside it. The host
            # supplies the identical layout, so each block is ONE contiguous
            # 2D transfer with multi-KB rows.
            xall = xp.tile([128, NHT * cap], bf16, name="xall")

            def xsl(i, off, sz):  # moving operand [128, sz] for h-tile i
                base = NHT * off + i * sz
                return xall[:, base:base + sz]

            # Output accumulator (resident, f32), same column layout as xall.
            yall = yp.tile([128, NHT * cap], f32, name="yall")

            def ysl(i, off, sz):
                return yall[:, i * cap + off:i * cap + off + sz]

            # Prologue. Token block 0 (one contiguous ~1MB 2D transfer) then
            # the remaining blocks ride the scalar HWDGE queue while the first
            # f-chunk's weights stream on the sync queue in parallel, first w1
            # piece first. The PE starts once block 0 and w1's first piece
            # land (~11us) and never waits again.
            _, sz0 = blocks[0]
            nc.scalar.dma_start(xall[:, 0:NHT * sz0], xT[:, 0:NHT * sz0])
            w1c = wp.tile([128, NFT * H], bf16, tag="w1", name="w1c")
            w2c = wp.tile([128, NFT * H], bf16, tag="w2", name="w2c")
            for j in range(NFT):
                jsl = slice(j * H, (j + 1) * H)
                nc.sync.dma_start(w1c[:, jsl], w1[0][:, jsl])
                nc.scalar.dma_start(w2c[:, jsl], w2[0][:, jsl])
            w3c = wp.tile([128, NFT * H], bf16, tag="w3", name="w3c")
            nc.sync.dma_start(w3c[:], w3[0])
            if len(blocks) > 1:
                # remaining token blocks, in consumption order
                for off, sz in blocks[1:]:
                    lo, hi = NHT * off, NHT * (off + sz)
                    nc.scalar.dma_start(xall[:, lo:hi], xT[:, lo:hi])

            # PE warmup during the prologue DMA wait: ~5us of tiny matmuls
            # on the bias tile so the HAM clock-gate opens before the first
            # real matmul. Result lands in yall[0:32, 0:8], which fc0's
            # stage-B eviction overwrites.
            warm = pg.tile([128, 32], f32, tag="warm", name="warm")
            nwarm = 48
            for w in range(nwarm):
                nc.tensor.matmul(
                    warm[0:32, 0:NHT], b1t[:, 0:32], b1t[:, 0:NHT],
                    start=(w == 0), stop=(w == nwarm - 1),
                )
            nc.vector.tensor_copy(yall[0:32, 0:NHT], warm[0:32, 0:NHT])

            for fc in range(NFCH):
                if fc > 0:
                    # Stream this f-chunk's weights (each byte loaded once).
                    w1c = wp.tile([128, NFT * H], bf16, tag="w1", name="w1c")
                    nc.sync.dma_start(w1c[:], w1[fc])
                    w2c = wp.tile([128, NFT * H], bf16, tag="w2", name="w2c")
                    nc.sync.dma_start(w2c[:], w2[fc])
                    w3c = wp.tile([128, NFT * H], bf16, tag="w3", name="w3c")
                    nc.sync.dma_start(w3c[:], w3[fc])

                def stage_b(off, sz, ht_tiles):
                    # Stage B: y_T[h, tok] += w3_chunk.T @ h_T
                    # w3c columns: (j, h) -> f-tile j, output col h.
                    for i in range(NHT):
                        psy = py.tile([128, sz], f32, tag="y", name="psy")
                        for j in range(NFT):
                            nc.tensor.matmul(
                                psy[:],
                                w3c[:, j * H + i * 128:j * H + (i + 1) * 128],
                                ht_tiles[j][:],
                                start=(j == 0), stop=(j == NFT - 1),
                            )
                        if fc == 0:
                            nc.scalar.activation(
                                ysl(i, off, sz), psy[:], AF.Identity,
                                bias=b3t[:, i:i + 1],
                            )
                        else:
                            nc.vector.tensor_add(
                                ysl(i, off, sz), ysl(i, off, sz), psy[:]
                            )
                    if fc == NFCH - 1:
                        for i in range(NHT):
                            lo, hi = i * cap + off, i * cap + off + sz
                            nc.sync.dma_start(yT[:, lo:hi], yall[:, lo:hi])

                pending = None  # (off, sz, ht_tiles) awaiting stage B
                for off, sz in blocks:
                    # Stage A: h_T[f, tok] = silu(g_T + b1) * (u_T + b2)
                    # w1c/w2c columns: (j, i, q) -> f-tile j, h-tile i, col q.
                    ht_tiles = []
                    for j in range(NFT):
                        fg = fc * NFT + j  # global f-tile index
                        psg = pg.tile([128, sz], f32, tag="g", name="psg")
                        for i in range(NHT):
                            base = (j * NHT + i) * 128
                            nc.tensor.matmul(
                                psg[:], w1c[:, base:base + 128], xsl(i, off, sz),
                                start=(i == 0), stop=(i == NHT - 1),
                            )
                        s = sp.tile([128, sz], f32, tag="s", name="stile")
                        nc.scalar.activation(
                            s[:], psg[:], AF.Silu, bias=b1t[:, fg:fg + 1]
                        )
                        psu = pu.tile([128, sz], f32, tag="u", name="psu")
                        for i in range(NHT):
                            base = (j * NHT + i) * 128
                            nc.tensor.matmul(
                                psu[:], w2c[:, base:base + 128], xsl(i, off, sz),
                                start=(i == 0), stop=(i == NHT - 1),
                            )
                        h = hp.tile([128, sz], bf16, tag=f"h{j}", name=f"htile{j}")
                        if use_b2:
                            u2 = sp.tile([128, sz], f32, tag="u2", name="u2tile")
                            nc.scalar.activation(
                                u2[:], psu[:], AF.Identity, bias=b2t[:, fg:fg + 1]
                            )
                            nc.vector.tensor_mul(h[:], s[:], u2[:])
                        else:
                            nc.vector.tensor_mul(h[:], s[:], psu[:])
                        ht_tiles.append(h)

                    if pending is not None:
                        stage_b(*pending)
                    pending = (off, sz, ht_tiles)
                stage_b(*pending)

    nc.finalize()
    return nc


def _route(x2d: np.ndarray, router_w: np.ndarray):
    """Host router: softmax over experts, top-2. Returns per-expert token
    index lists and combine weights."""
    logits = x2d @ router_w                       # [T, E]
    logits -= logits.max(axis=-1, keepdims=True)
    p = np.exp(logits, dtype=np.float32)
    p /= p.sum(axis=-1, keepdims=True)
    # top-2 expert ids per token (ties: lower index first, like lax.top_k)
    order = np.argsort(-p, axis=-1, kind="stable")[:, :K]   # [T, K]
    idx_e, cw_e = [], []
    for e in range(E):
        sel = np.nonzero((order == e).any(axis=1))[0]
        idx_e.append(sel)
        cw_e.append(p[sel, e])
    return idx_e, cw_e


def _pack_w12(w: np.ndarray) -> np.ndarray:
    """[H, F] f32 -> [NFCH, 128, NFT*NHT*128] bf16 with column order (j, i, q):
    chunk c, partition p, f-tile j, h-tile i, col q = w[i*128+p, c*FCH+j*128+q].
    """
    t = np.asarray(w, dtype=np.float32).reshape(NHT, 128, NFCH, NFT, 128)
    t = t.transpose(2, 1, 3, 0, 4)  # [c, p, j, i, q]
    return np.ascontiguousarray(t.astype(_BF16)).reshape(NFCH, 128, NFT * H)


def _pack_w3(w: np.ndarray) -> np.ndarray:
    """[F, H] f32 -> [NFCH, 128, NFT*H] bf16 with column order (j, h):
    chunk c, partition p (= f within f-tile j) -> w[c*FCH+j*128+p, h]."""
    t = np.asarray(w, dtype=np.float32).reshape(NFCH, NFT, 128, H)
    t = t.transpose(0, 2, 1, 3)  # [c, p, j, h]
    return np.ascontiguousarray(t.astype(_BF16)).reshape(NFCH, 128, NFT * H)


def kernel(x, router_w, w1, b1, w2, b2, w3, b3):
    from concourse.bass_utils import run_bass_kernel_spmd

    B, S, _ = x.shape
    T = B * S
    x2d = np.ascontiguousarray(x, dtype=np.float32).reshape(T, H)

    idx_e, cw_e = _route(x2d, np.asarray(router_w, dtype=np.float32))
    max_n = max(len(i) for i in idx_e)
    blocks, cap = _blocks_for(max_n)

    use_b2 = bool(np.any(b2))
    key = (tuple(blocks), use_b2)
    nc = _kernel_cache.get(key)
    if nc is None:
        nc = _build(blocks, use_b2)
        _kernel_cache[key] = nc

    in_maps = []
    for e in range(E):
        idx = idx_e[e]
        xg = np.zeros((cap, H), dtype=np.float32)
        xg[: len(idx)] = x2d[idx]
        # [cap, H] -> [128, NHT*cap], block-major columns: block at token
        # offset `off` spans cols [NHT*off, NHT*(off+sz)), h-tile i contiguous
        # inside it: col = NHT*off + i*sz + t.
        xb = xg.astype(_BF16)
        xTe = np.concatenate(
            [
                xb[off:off + sz].reshape(sz, NHT, 128)
                .transpose(2, 1, 0).reshape(128, NHT * sz)
                for off, sz in blocks
            ],
            axis=1,
        )
        xTe = np.ascontiguousarray(xTe)
        m = {
            "xT": xTe,
            "w1": _pack_w12(w1[e]),
            "w2": _pack_w12(w2[e]),
            "w3": _pack_w3(w3[e]),
            "b1": np.ascontiguousarray(
                np.asarray(b1[e], dtype=np.float32).reshape(F // 128, 128).T
            ),
            "b3": np.ascontiguousarray(
                np.asarray(b3[e], dtype=np.float32).reshape(NHT, 128).T
            ),
        }
        if use_b2:
            m["b2"] = np.ascontiguousarray(
                np.asarray(b2[e], dtype=np.float32).reshape(F // 128, 128).T
            )
        in_maps.append(m)

    global _last_in_maps
    _last_in_maps = in_maps
    res = run_bass_kernel_spmd(nc, in_maps, core_ids=list(range(E)))

    out = np.zeros((T, H), dtype=np.float32)
    for e in range(E):
        idx = idx_e[e]
        n = len(idx)
        # yT [128, NHT*cap] -> y[t, h]: y[t, i*128+p] = yT[p, i*cap+t]
        yTe = res.results[e]["yT"].reshape(128, NHT, cap)
        ye = yTe[:, :, :n].transpose(2, 1, 0).reshape(n, H)
        out[idx] += ye * cw_e[e][:, None]
    return out.reshape(B, S, H)


# revision 22
# speedup vs baseline: 1.0042x; 1.0042x over previous
"""MoE FFN (top-2 of 8 experts, SwiGLU) for 8 Trainium2 NeuronCores.

Strategy: expert parallelism. The router (tiny [T,H]@[H,E] matmul + softmax +
top-2) runs on host as part of sharding; tokens are dispatched ("alltoall by
routing decision") to the core owning their expert. Each core runs a dense
SwiGLU FFN over its gathered tokens in bf16 (fp32 PSUM accumulation), in a
feature-on-partition / token-on-free-dim layout so no on-device transposes are
needed and every weight byte is DMA'd exactly once, as a handful of large
contiguous transfers. The host applies the combine weights and scatter-adds
the per-expert outputs into the full output.

Per-core device program (expert e), with nht = H/128 h-tiles, f-chunks of
FCH columns (nft f-tiles each):
  g_T[f, t] = sum_i w1[h_i, f]^T @ x_T[h_i, t]        (PSUM accum over h-tiles)
  u_T[f, t] likewise with w2
  h_T[f, t] = silu(g_T + b1) * (u_T + b2)             (ACT + DVE, -> bf16)
  y_T[h, t] = sum_f w3[f, h]^T @ h_T[f, t] + b3       (PSUM accum per f-chunk,
                                                       accumulated in SBUF f32)
Weights stream through SBUF one f-chunk at a time; tokens/outputs are SBUF-
resident. Every matmul has a 128-row stationary operand in natural layout and
a [128, block] moving operand, so the PE runs back-to-back at stream rate.
"""

import numpy as np
import ml_dtypes

E = 8       # experts == cores
K = 2       # top-k
H = 1024    # hidden
F = 4096    # ffn dim
BLK = 512   # max tokens per block (moving free dim of every matmul)
FCH = 512   # f-chunk size (weight streaming granularity); FCH % 128 == 0

NHT = H // 128    # h-tiles
NFCH = F // FCH   # f-chunks
NFT = FCH // 128  # f-tiles per chunk

_BF16 = ml_dtypes.bfloat16

_kernel_cache: dict[object, object] = {}
_last_in_maps = None


def _blocks_for(max_n: int):
    """Token-block sizes covering max_n tokens: full 512-blocks plus a small
    tail block, so padded capacity hugs the real max expert load."""
    max_n = max(max_n, 16)
    nfull, rem = divmod(max_n, BLK)
    rem = (rem + 7) // 8 * 8  # keep DMA rows 16B-aligned
    sizes = [BLK] * nfull + ([rem] if rem else [])
    blocks = []
    off = 0
    for sz in sizes:
        blocks.append((off, sz))
        off += sz
    return blocks, off


def _build(blocks, use_b2: bool):
    """Build the per-core Bass/Tile program for the given token blocks."""
    import concourse.bass as bass  # noqa: F401
    import concourse.tile as tile
    from concourse import bacc, mybir

    bf16 = mybir.dt.bfloat16
    f32 = mybir.dt.float32
    AF = mybir.ActivationFunctionType

    cap = sum(sz for _, sz in blocks)

    nc = bacc.Bacc("TRN2", target_bir_lowering=False, debug=False, num_devices=E)

    # Host-side layouts are chosen so every DMA is a large 2D/3D transfer with
    # long contiguous rows (see kernel() for the packing).
    xT = nc.declare_dram_parameter("xT", [128, NHT * cap], bf16, isOutput=False)
    w1 = nc.declare_dram_parameter("w1", [NFCH, 128, NFT * H], bf16, isOutput=False)
    w2 = nc.declare_dram_parameter("w2", [NFCH, 128, NFT * H], bf16, isOutput=False)
    w3 = nc.declare_dram_parameter("w3", [NFCH, 128, NFT * H], bf16, isOutput=False)
    b1 = nc.declare_dram_parameter("b1", [128, F // 128], f32, isOutput=False)
    b3 = nc.declare_dram_parameter("b3", [128, NHT], f32, isOutput=False)
    if use_b2:
        b2 = nc.declare_dram_parameter("b2", [128, F // 128], f32, isOutput=False)
    yT = nc.declare_dram_parameter("yT", [128, NHT * cap], f32, isOutput=True)

    with tile.TileContext(nc) as tc:
        with (
            tc.tile_pool(name="xp", bufs=1) as xp,
            tc.tile_pool(name="yp", bufs=1) as yp,
            tc.tile_pool(name="wp", bufs=2) as wp,
            tc.tile_pool(name="hp", bufs=2) as hp,
            tc.tile_pool(name="sp", bufs=3) as sp,
            tc.tile_pool(name="bp", bufs=1) as bp,
            tc.tile_pool(name="pg", bufs=2, space="PSUM") as pg,
            tc.tile_pool(name="pu", bufs=2, space="PSUM") as pu,
            tc.tile_pool(name="py", bufs=2, space="PSUM") as py,
        ):
            # Biases (tiny, resident)
            b1t = bp.tile([128, F // 128], f32, tag="b1", name="b1t")
            nc.sync.dma_start(b1t[:], b1[:])
            b3t = bp.tile([128, NHT], f32, tag="b3", name="b3t")
            nc.sync.dma_start(b3t[:], b3[:])
            if use_b2:
                b2t = bp.tile([128, F // 128], f32, tag="b2", name="b2t")
                nc.sync.dma_start(b2t[:], b2[:])

            # Tokens (resident, bf16): one [128, NHT*cap] tile in BLOCK-major
            # column order — token block at offset `off` occupies columns
            # [NHT*off, NHT*(off+sz)), h-tile i contiguous inside it. The host
            # supplies the identical layout, so each block is ONE contiguous
            # 2D transfer with multi-KB rows.
            xall = xp.tile([128, NHT * cap], bf16, name="xall")

            def xsl(i, off, sz):  # moving operand [128, sz] for h-tile i
                base = NHT * off + i * sz
                return xall[:, base:base + sz]

            # Output accumulator (resident, f32), same column layout as xall.
            yall = yp.tile([128, NHT * cap], f32, name="yall")

            def ysl(i, off, sz):
                return yall[:, i * cap + off:i * cap + off + sz]

            # Prologue. Token block 0 (one contiguous ~1MB 2D transfer) then
            # the remaining blocks ride the scalar HWDGE queue while the first
            # f-chunk's weights stream on the sync queue in parallel, first w1
            # piece first. The PE starts once block 0 and w1's first piece
            # land (~11us) and never waits again.
            _, sz0 = blocks[0]
            nc.scalar.dma_start(xall[:, 0:NHT * sz0], xT[:, 0:NHT * sz0])
            w1c = wp.tile([128, NFT * H], bf16, tag="w1", name="w1c")
            w2c = wp.tile([128, NFT * H], bf16, tag="w2", name="w2c")
            for j in range(NFT):
                jsl = slice(j * H, (j + 1) * H)
                nc.sync.dma_start(w1c[:, jsl], w1[0][:, jsl])
                nc.scalar.dma_start(w2c[:, jsl], w2[0][:, jsl])
            w3c = wp.tile([128, NFT * H], bf16, tag="w3", name="w3c")
            nc.sync.dma_start(w3c[:], w3[0])
            if len(blocks) > 1:
                # remaining token blocks, in consumption order
                for off, sz in blocks[1:]:
                    lo, hi = NHT * off, NHT * (off + sz)
                    nc.scalar.dma_start(xall[:, lo:hi], xT[:, lo:hi])

            for fc in range(NFCH):
                if fc > 0:
                    # Stream this f-chunk's weights (each byte loaded once).
                    w1c = wp.tile([128, NFT * H], bf16, tag="w1", name="w1c")
                    nc.sync.dma_start(w1c[:], w1[fc])
                    w2c = wp.tile([128, NFT * H], bf16, tag="w2", name="w2c")
                    nc.sync.dma_start(w2c[:], w2[fc])
                    w3c = wp.tile([128, NFT * H], bf16, tag="w3", name="w3c")
                    nc.sync.dma_start(w3c[:], w3[fc])

                def stage_b(off, sz, ht_tiles):
                    # Stage B: y_T[h, tok] += w3_chunk.T @ h_T
                    # w3c columns: (j, h) -> f-tile j, output col h.
                    for i in range(NHT):
                        psy = py.tile([128, sz], f32, tag="y", name="psy")
                        for j in range(NFT):
                            nc.tensor.matmul(
                                psy[:],
                                w3c[:, j * H + i * 128:j * H + (i + 1) * 128],
                                ht_tiles[j][:],
                                start=(j == 0), stop=(j == NFT - 1),
                            )
                        if fc == 0:
                            nc.scalar.activation(
                                ysl(i, off, sz), psy[:], AF.Identity,
                                bias=b3t[:, i:i + 1],
                            )
                        else:
                            nc.vector.tensor_add(
                                ysl(i, off, sz), ysl(i, off, sz), psy[:]
                            )
                    if fc == NFCH - 1:
                        for i in range(NHT):
                            lo, hi = i * cap + off, i * cap + off + sz
                            nc.sync.dma_start(yT[:, lo:hi], yall[:, lo:hi])

                pending = None  # (off, sz, ht_tiles) awaiting stage B
                for off, sz in blocks:
                    # Stage A: h_T[f, tok] = silu(g_T + b1) * (u_T + b2)
                    # w1c/w2c columns: (j, i, q) -> f-tile j, h-tile i, col q.
                    ht_tiles = []
                    for j in range(NFT):
                        fg = fc * NFT + j  # global f-tile index
                        psg = pg.tile([128, sz], f32, tag="g", name="psg")
                        for i in range(NHT):
                            base = (j * NHT + i) * 128
                            nc.tensor.matmul(
                                psg[:], w1c[:, base:base + 128], xsl(i, off, sz),
                                start=(i == 0), stop=(i == NHT - 1),
                            )
                        s = sp.tile([128, sz], f32, tag="s", name="stile")
                        nc.scalar.activation(
                            s[:], psg[:], AF.Silu, bias=b1t[:, fg:fg + 1]
                        )
                        psu = pu.tile([128, sz], f32, tag="u", name="psu")
                        for i in range(NHT):
                            base = (j * NHT + i) * 128
                            nc.tensor.matmul(
                                psu[:], w2c[:, base:base + 128], xsl(i, off, sz),
                                start=(i == 0), stop=(i == NHT - 1),
                            )
                        h = hp.tile([128, sz], bf16, tag=f"h{j}", name=f"htile{j}")
                        if use_b2:
                            u2 = sp.tile([128, sz], f32, tag="u2", name="u2tile")
                            nc.scalar.activation(
                                u2[:], psu[:], AF.Identity, bias=b2t[:, fg:fg + 1]
                            )
                            nc.vector.tensor_mul(h[:], s[:], u2[:])
                        else:
                            nc.vector.tensor_mul(h[:], s[:], psu[:])
                        ht_tiles.append(h)

                    if pending is not None:
                        stage_b(*pending)
                    pending = (off, sz, ht_tiles)
                stage_b(*pending)

    nc.finalize()
    return nc


def _route(x2d: np.ndarray, router_w: np.ndarray):
    """Host router: softmax over experts, top-2. Returns per-expert token
    index lists and combine weights."""
    logits = x2d @ router_w                       # [T, E]
    logits -= logits.max(axis=-1, keepdims=True)
    p = np.exp(logits, dtype=np.float32)
    p /= p.sum(axis=-1, keepdims=True)
    # top-2 expert ids per token (ties: lower index first, like lax.top_k)
    order = np.argsort(-p, axis=-1, kind="stable")[:, :K]   # [T, K]
    idx_e, cw_e = [], []
    for e in range(E):
        sel = np.nonzero((order == e).any(axis=1))[0]
        idx_e.append(sel)
        cw_e.append(p[sel, e])
    return idx_e, cw_e


def _pack_w12(w: np.ndarray) -> np.ndarray:
    """[H, F] f32 -> [NFCH, 128, NFT*NHT*128] bf16 with column order (j, i, q):
    chunk c, partition p, f-tile j, h-tile i, col q = w[i*128+p, c*FCH+j*128+q].
    """
    t = np.asarray(w, dtype=np.float32).reshape(NHT, 128, NFCH, NFT, 128)
    t = t.transpose(2, 1, 3, 0, 4)  # [c, p, j, i, q]
    return np.ascontiguousarray(t.astype(_BF16)).reshape(NFCH, 128, NFT * H)


def _pack_w3(w: np.ndarray) -> np.ndarray:
    """[F, H] f32 -> [NFCH, 128, NFT*H] bf16 with column order (j, h):
    chunk c, partition p (= f within f-tile j) -> w[c*FCH+j*128+p, h]."""
    t = np.asarray(w, dtype=np.float32).reshape(NFCH, NFT, 128, H)
    t = t.transpose(0, 2, 1, 3)  # [c, p, j, h]
    return np.ascontiguousarray(t.astype(_BF16)).reshape(NFCH, 128, NFT * H)


def kernel(x, router_w, w1, b1, w2, b2, w3, b3):
    from concourse.bass_utils import run_bass_kernel_spmd

    B, S, _ = x.shape
    T = B * S
    x2d = np.ascontiguousarray(x, dtype=np.float32).reshape(T, H)

    idx_e, cw_e = _route(x2d, np.asarray(router_w, dtype=np.float32))
    max_n = max(len(i) for i in idx_e)
    blocks, cap = _blocks_for(max_n)

    use_b2 = bool(np.any(b2))
    key = (tuple(blocks), use_b2)
    nc = _kernel_cache.get(key)
    if nc is None:
        nc = _build(blocks, use_b2)
        _kernel_cache[key] = nc

    in_maps = []
    for e in range(E):
        idx = idx_e[e]
        xg = np.zeros((cap, H), dtype=np.float32)
        xg[: len(idx)] = x2d[idx]
        # [cap, H] -> [128, NHT*cap], block-major columns: block at token
        # offset `off` spans cols [NHT*off, NHT*(off+sz)), h-tile i contiguous
        # inside it: col = NHT*off + i*sz + t.
        xb = xg.astype(_BF16)
        xTe = np.concatenate(
            [
                xb[off:off + sz].reshape(sz, NHT, 128)
                .transpose(2, 1, 0).reshape(128, NHT * sz)
                for off, sz in blocks
            ],
            axis=1,
        )
        xTe = np.ascontiguousarray(xTe)
        m = {
            "xT": xTe,
            "w1": _pack_w12(w1[e]),
            "w2": _pack_w12(w2[e]),
            "w3": _pack_w3(w3[e]),
            "b1": np.ascontiguousarray(
                np.asarray(b1[e], dtype=np.float32).reshape(F // 128, 128).T
            ),
            "b3": np.ascontiguousarray(
                np.asarray(b3[e], dtype=np.float32).reshape(NHT, 128).T
            ),
        }
        if use_b2:
            m["b2"] = np.ascontiguousarray(
                np.asarray(b2[e], dtype=np.float32).reshape(F // 128, 128).T
            )
        in_maps.append(m)

    global _last_in_maps
    _last_in_maps = in_maps
    res = run_bass_kernel_spmd(nc, in_maps, core_ids=list(range(E)))

    out = np.zeros((T, H), dtype=np.float32)
    for e in range(E):
        idx = idx_e[e]
        n = len(idx)
        # yT [128, NHT*cap] -> y[t, h]: y[t, i*128+p] = yT[p, i*cap+t]
        yTe = res.results[e]["yT"].reshape(128, NHT, cap)
        ye = yTe[:, :, :n].transpose(2, 1, 0).reshape(n, H)
        out[idx] += ye * cw_e[e][:, None]
    return out.reshape(B, S, H)
